# revision 1
# baseline (speedup 1.0000x reference)
"""Trainium2 Bass kernel for nn_Attention_10797547782838.

Windowed multi-head attention with per-query angle bias:
  q = (x@Wq+bq) reshaped to heads; k,v = x@Wkv+bkv
  attn = softmax((q*scale)@k^T * anglebias(q) + mask[b%4]); out = attn@v@Wp.

Sharding: batch data-parallel over 8 cores; core m gets batches {m, m+8},
which share the same window mask (b%4 == m%4).

Device algorithm (per batch):
  - S^T layout [keys, queries]; angle bias and scale folded into q on host.
  - exp(S+M) computed three ways to balance engines (chunk-assigned):
      A: ACT exp(S) -> bf16, DVE multiply by em=exp(M) (bf16)
      B: DVE scalar_tensor_tensor Schraudolph: int16(S*a + M') bitcast bf16
      C: same as B on GPSIMD
    where M' = rint(a*M + 16256 + c) is an int16 host-precomputed tile, so
    the Schraudolph paths get the mask add for free.
  - PV matmul uses [V_h | 1] 33-wide stationaries: partition 32 of each
    output strip accumulates the softmax denominator, eliminating the
    separate ones-matmul entirely.
  - Denominator rows are DMA-gathered from PSUM, inverted with
    reciprocal_approx_fast, broadcast back through an 8x128 selector
    matmul, and applied to O^T on DVE before the output projection.
  - Output projection uses per-strip Wp with zeroed junk rows so the
    33-partition strips need no compaction.
"""
import math
import sys

import numpy as np

sys.path.insert(0, "/opt/trn_rl_repo")

import concourse.bass as bass  # noqa: E402
import concourse.bacc as bacc  # noqa: E402
import concourse.tile as tile  # noqa: E402
from concourse import mybir  # noqa: E402
from concourse.bass_utils import run_bass_kernel_spmd  # noqa: E402

F32 = mybir.dt.float32
F32R = mybir.dt.float32r
BF16 = mybir.dt.bfloat16
I16 = mybir.dt.int16
AF = mybir.ActivationFunctionType
ALU = mybir.AluOpType

B, N, C = 16, 1024, 256
HEADS, HD = 8, 32
NW = 4
N_CORES = 8
BPC = B // N_CORES  # batches per core

AA = 184.6650390625        # 128 * log2(e)
MAGIC_C = -11.0            # Schraudolph rounding correction

# per-batch chunk counts for exp paths (64 chunks of [128,1024] per batch)
# A: ACT exp + DVE mask-mult; G: ACT exp + GPSIMD mask-mult; B: DVE
# Schraudolph scalar_tensor_tensor (GPSIMD cannot read PSUM)
N_A, N_G, N_B = 31, 21, 12

_CACHE = {}


def r32(ap):
    return ap.bitcast(F32R)


def bc4(a):
    # insert a stride-0 broadcast dim after the partition dim: [p, n] -> [p, 4, n]
    return bass.AP(a.tensor, a.offset, [a.ap[0], [0, 4]] + list(a.ap[1:]))


def make_pattern(na, ng, nb):
    tgt = {"A": na, "G": ng, "B": nb}
    cnt = {"A": 0, "G": 0, "B": 0}
    pat = []
    for _ in range(na + ng + nb):
        best, bscore = None, None
        for p in ("A", "G", "B"):
            if tgt[p] == 0 or cnt[p] >= tgt[p]:
                continue
            score = (cnt[p] + 1.0) / tgt[p]
            if bscore is None or score < bscore:
                best, bscore = p, score
        cnt[best] += 1
        pat.append(best)
    return pat


PATTERN = make_pattern(N_A, N_G, N_B)


def build_kernel():
    from contextlib import ExitStack
    nc = bacc.Bacc("TRN2", target_bir_lowering=False, debug=False,
                   num_devices=N_CORES)

    d_xT = nc.dram_tensor("xT", [BPC, C, N], BF16, kind="ExternalInput").ap()
    d_biasT = nc.dram_tensor("biasT", [BPC, C, N], F32, kind="ExternalInput").ap()
    d_em = nc.dram_tensor("em", [N, N], BF16, kind="ExternalInput").ap()
    d_mp = nc.dram_tensor("mp", [N, N], I16, kind="ExternalInput").ap()
    d_wq = nc.dram_tensor("wq", [C, C], BF16, kind="ExternalInput").ap()
    d_wk = nc.dram_tensor("wk", [C, C], BF16, kind="ExternalInput").ap()
    d_wv = nc.dram_tensor("wv", [C, C], BF16, kind="ExternalInput").ap()
    d_wp = nc.dram_tensor("wp", [4, 128, C], BF16, kind="ExternalInput").ap()
    d_bq = nc.dram_tensor("bq", [128, 2], F32, kind="ExternalInput").ap()
    d_bk = nc.dram_tensor("bk", [128, 2], F32, kind="ExternalInput").ap()
    d_bv = nc.dram_tensor("bv", [128, C], F32, kind="ExternalInput").ap()
    d_bp = nc.dram_tensor("bp", [128, C], F32, kind="ExternalInput").ap()
    d_sel = nc.dram_tensor("sel", [2, 128], BF16, kind="ExternalInput").ap()
    d_y = nc.dram_tensor("y", [BPC, N, C], F32, kind="ExternalOutput").ap()

    with tile.TileContext(nc) as tc:
        with ExitStack() as ctx, nc.allow_low_precision(reason="bf16/f32r matmul inputs; fp32 PSUM accumulation; approx softmax exp within tolerance"):
            kernel_body(ctx, tc, d_xT, d_biasT, d_em, d_mp, d_wq, d_wk, d_wv,
                        d_wp, d_bq, d_bk, d_bv, d_bp, d_sel, d_y)
    nc.compile()
    return nc


def kernel_body(ctx, tc, d_xT, d_biasT, d_em, d_mp, d_wq, d_wk, d_wv, d_wp,
                d_bq, d_bk, d_bv, d_bp, d_sel, d_y):
    nc = tc.nc

    consts = ctx.enter_context(tc.tile_pool(name="consts", bufs=1))
    xpool = ctx.enter_context(tc.tile_pool(name="xpool", bufs=2))
    qkv = ctx.enter_context(tc.tile_pool(name="qkv", bufs=2))
    vpool = ctx.enter_context(tc.tile_pool(name="vpool", bufs=2))
    ppool = ctx.enter_context(tc.tile_pool(name="ppool", bufs=15))
    otpool = ctx.enter_context(tc.tile_pool(name="otpool", bufs=2))
    misc = ctx.enter_context(tc.tile_pool(name="misc", bufs=2))
    ypool = ctx.enter_context(tc.tile_pool(name="ypool", bufs=3))
    psS = ctx.enter_context(tc.tile_pool(name="psS", bufs=2, space="PSUM"))
    psO = ctx.enter_context(tc.tile_pool(name="psO", bufs=1, space="PSUM"))

    # ---- constants / weights (once) ----
    w_sb = {}
    for name, dram in (("wq", d_wq), ("wk", d_wk), ("wv", d_wv)):
        t = consts.tile([128, 2, C], BF16, tag=f"w_{name}")
        nc.sync.dma_start(out=t[:], in_=dram.rearrange("(c p) n -> p c n", p=128))
        w_sb[name] = t
    # wp comes pre-permuted per otn strip: [4 strips, 128 rows, C]
    wp_sb = consts.tile([128, 4, C], BF16, tag="w_wp")
    nc.sync.dma_start(out=wp_sb[:], in_=d_wp.rearrange("s p n -> p s n"))
    bq_sb = consts.tile([128, 2], F32, tag="bq")
    nc.sync.dma_start(out=bq_sb[:], in_=d_bq[:])
    bk_sb = consts.tile([128, 2], F32, tag="bk")
    nc.sync.dma_start(out=bk_sb[:], in_=d_bk[:])
    bv_sb = consts.tile([128, C], F32, tag="bv")
    nc.sync.dma_start(out=bv_sb[:], in_=d_bv[:])
    bp_sb = consts.tile([128, C], F32, tag="bp")
    nc.sync.dma_start(out=bp_sb[:], in_=d_bp[:])

    # masks staged once per core (both batches share b%4); DMAs issued
    # after the projection inputs so the first matmuls start sooner
    em_sb = [consts.tile([128, N], BF16, tag=f"em{kc}", name=f"em{kc}")
             for kc in range(8)]
    mp_sb = [consts.tile([128, N], I16, tag=f"mp{kc}", name=f"mp{kc}")
             for kc in range(8)]

    # selector stationaries for denominator broadcast: selA covers
    # partitions 0:64, selB covers 64:128 (host-built patterns)
    selA = consts.tile([1, 128], BF16, tag="selA")
    nc.sync.dma_start(out=selA[:], in_=d_sel[0:1, :])
    selB = consts.tile([1, 128], BF16, tag="selB")
    nc.sync.dma_start(out=selB[:], in_=d_sel[1:2, :])

    # ---- stage inputs and run projections for BOTH batches ----
    xT_sb, biasT_sb, qT_sb, kT_sb, v_sb = {}, {}, {}, {}, {}
    for b in range(BPC):
        xT_sb[b] = xpool.tile([128, 2, N], BF16, tag="xT", name=f"xT{b}")
        nc.sync.dma_start(out=xT_sb[b][:],
                          in_=d_xT[b].rearrange("(c p) n -> p c n", p=128))
        biasT_sb[b] = xpool.tile([128, 2, N], F32, tag="biasT", name=f"bT{b}")
        nc.sync.dma_start(out=biasT_sb[b][:],
                          in_=d_biasT[b].rearrange("(c p) n -> p c n", p=128))

    for kc in range(8):
        nc.sync.dma_start(out=em_sb[kc][:], in_=d_em[kc * 128:(kc + 1) * 128, :])
        nc.sync.dma_start(out=mp_sb[kc][:], in_=d_mp[kc * 128:(kc + 1) * 128, :])

    for b in range(BPC):
        qT_sb[b] = qkv.tile([128, 2, N], BF16, tag="qT", name=f"qT{b}")
        kT_sb[b] = qkv.tile([128, 2, N], BF16, tag="kT", name=f"kT{b}")
        for m in range(2):
            for qc in range(2):
                ps_q = psS.tile([128, 512], F32, tag="s")
                for ci in range(2):
                    nc.tensor.matmul(
                        ps_q[:],
                        w_sb["wq"][:, ci, m * 128:(m + 1) * 128],
                        xT_sb[b][:, ci, qc * 512:(qc + 1) * 512],
                        start=(ci == 0), stop=(ci == 1))
                nc.vector.scalar_tensor_tensor(
                    out=qT_sb[b][:, m, qc * 512:(qc + 1) * 512], in0=ps_q[:],
                    scalar=bq_sb[:, m:m + 1],
                    in1=biasT_sb[b][:, m, qc * 512:(qc + 1) * 512],
                    op0=ALU.add, op1=ALU.mult)
                ps_k = psS.tile([128, 512], F32, tag="s")
                for ci in range(2):
                    nc.tensor.matmul(
                        ps_k[:],
                        w_sb["wk"][:, ci, m * 128:(m + 1) * 128],
                        xT_sb[b][:, ci, qc * 512:(qc + 1) * 512],
                        start=(ci == 0), stop=(ci == 1))
                nc.vector.tensor_scalar_add(
                    out=kT_sb[b][:, m, qc * 512:(qc + 1) * 512], in0=ps_k[:],
                    scalar1=bk_sb[:, m:m + 1])

        v_sb[b] = []
        for t8 in range(8):
            vt = vpool.tile([128, 8, 33], BF16, tag=f"v{t8}", name=f"v{b}_{t8}")
            ps_v = psS.tile([128, C], F32, tag="s")
            for ci in range(2):
                nc.tensor.matmul(
                    ps_v[:],
                    xT_sb[b][:, ci, t8 * 128:(t8 + 1) * 128],
                    w_sb["wv"][:, ci, :],
                    start=(ci == 0), stop=(ci == 1))
            if t8 % 2 == 0:
                nc.scalar.copy(out=vt[:, :, 0:32], in_=ps_v[:])
            else:
                nc.vector.tensor_tensor(
                    out=vt[:, :, 0:32], in0=ps_v[:], in1=bv_sb[:], op=ALU.add)
            nc.gpsimd.memset(vt[:, :, 32:33], 1.0)
            v_sb[b].append(vt)

    # ---- attention: batches interleaved; PV matmuls and tail pieces are
    # ---- emitted with a lag so the in-order PE queue never head-of-line
    # ---- blocks on the exp/mask path of the current chunk
    from collections import deque
    pending = deque()
    LAG = 12

    def step(fn):
        pending.append(fn)
        while len(pending) > LAG:
            pending.popleft()()

    chunk_i = {0: 0, 1: 0}
    for qc in range(2):
        for b in range(BPC):
            pot = psO.tile([128, 4, 512], F32, tag="po")
            for kc in range(8):
                for hp in range(4):  # head pair (2hp, 2hp+1)
                    path = PATTERN[chunk_i[b]]
                    chunk_i[b] += 1
                    ps_s = psS.tile([128, N], F32, tag="s")
                    for hh in range(2):
                        h = 2 * hp + hh
                        j = h % 4
                        nc.tensor.matmul(
                            ps_s[:, hh * 512:(hh + 1) * 512],
                            kT_sb[b][32 * j:32 * (j + 1), h // 4,
                                     kc * 128:(kc + 1) * 128],
                            qT_sb[b][32 * j:32 * (j + 1), h // 4,
                                     qc * 512:(qc + 1) * 512],
                            start=True, stop=True,
                            tile_position=(32 * j, 0))
                    emsl = em_sb[kc][:, qc * 512:(qc + 1) * 512]
                    mpsl = mp_sb[kc][:, qc * 512:(qc + 1) * 512]
                    if path in ("A", "G"):
                        p0 = ppool.tile([128, N], BF16, tag="p0")
                        pt = ppool.tile([128, N], BF16, tag="pt")
                        nc.scalar.activation(out=p0[:], in_=ps_s[:], func=AF.Exp)
                        eng = nc.vector if path == "A" else nc.gpsimd
                        for hh in range(2):
                            eng.tensor_tensor(
                                out=pt[:, hh * 512:(hh + 1) * 512],
                                in0=p0[:, hh * 512:(hh + 1) * 512],
                                in1=emsl, op=ALU.mult)
                        pt_bf = pt
                    else:
                        pti = ppool.tile([128, N], I16, tag="pt")
                        for hh in range(2):
                            nc.vector.scalar_tensor_tensor(
                                out=pti[:, hh * 512:(hh + 1) * 512],
                                in0=ps_s[:, hh * 512:(hh + 1) * 512],
                                scalar=AA, in1=mpsl,
                                op0=ALU.mult, op1=ALU.add)
                        pt_bf = pti.bitcast(BF16)

                    def po_mms(pot=pot, b=b, kc=kc, hp=hp, pt_bf=pt_bf):
                        for hh in range(2):
                            h = 2 * hp + hh
                            nc.tensor.matmul(
                                pot[64 * hh:64 * hh + 33, hp, :],
                                v_sb[b][kc][:, h, :],
                                pt_bf[:, hh * 512:(hh + 1) * 512],
                                start=(kc == 0), stop=(kc == 7),
                                tile_position=(0, 64 * hh))
                    step(po_mms)

            def tail(pot=pot, b=b, qc=qc):
                rs0 = misc.tile([1, 4, 512], BF16, tag="rs0")
                nc.scalar.copy(out=rs0[:], in_=pot[32:33, :, :])
                rs1 = misc.tile([1, 4, 512], BF16, tag="rs1")
                nc.vector.tensor_scalar_add(out=rs1[:],
                                            in0=pot[96:97, :, :], scalar1=0.0)
                otn = []
                for hp in range(4):
                    ps_rb = psS.tile([128, 512], F32, tag="s")
                    nc.tensor.matmul(ps_rb[:], selA[:], rs0[:, hp, :],
                                     start=True, stop=False)
                    nc.tensor.matmul(ps_rb[:], selB[:], rs1[:, hp, :],
                                     start=False, stop=True)
                    ssb = misc.tile([128, 512], F32, tag="ssb")
                    nc.scalar.copy(out=ssb[:], in_=ps_rb[:])
                    rbs = misc.tile([128, 512], F32, tag="rbs")
                    nc.vector.reciprocal_approx_fast(out=rbs[:], in_=ssb[:])
                    ot = otpool.tile([128, 512], BF16, tag=f"otn{hp}",
                                     name=f"ot{hp}")
                    nc.vector.tensor_tensor(
                        out=ot[:], in0=pot[:, hp, :], in1=rbs[:], op=ALU.mult)
                    otn.append(ot)
                for qt in range(4):
                    ps_y = psS.tile([128, C], F32, tag="s")
                    for hp in range(4):
                        nc.tensor.matmul(
                            ps_y[:],
                            otn[hp][:, qt * 128:(qt + 1) * 128],
                            wp_sb[:, hp, :],
                            start=(hp == 0), stop=(hp == 3))
                    y_sb = ypool.tile([128, C], F32, tag="y")
                    nc.scalar.copy(out=y_sb[:], in_=ps_y[:])
                    qoff = qc * 512 + qt * 128
                    nc.sync.dma_start(out=d_y[b, qoff:qoff + 128, :],
                                      in_=y_sb[:])
            step(tail)

    while pending:
        pending.popleft()()


def _host_prep(x, mask, affine_matrix, Wq, bq, Wkv, bkv, Wp, bp,
               angle_table, H, W):
    B_, N_, C_ = x.shape
    heads = angle_table.shape[1]
    hd = C_ // heads
    scale = np.float32(hd ** -0.5)
    H = int(H); W = int(W)

    gy, gx = np.meshgrid(np.arange(H, dtype=np.float32),
                         np.arange(W, dtype=np.float32), indexing="ij")
    coords = np.stack([gx.reshape(-1), gy.reshape(-1)], -1).astype(np.float32)
    center = np.array([W / 2.0, H / 2.0], np.float32)
    ego = np.einsum("bij,j->bi", affine_matrix[:, :2, :2], center) \
        + affine_matrix[:, :2, 2]
    rel = coords[None, :, :] - ego[:, None, :]
    ang = np.arctan2(rel[..., 1], rel[..., 0]).astype(np.float32)
    bins = (((ang + np.float32(math.pi)) / np.float32(2.0 * math.pi))
            * (angle_table.shape[0] - 1)).astype(np.int32)
    sig = (1.0 / (1.0 + np.exp(-angle_table[bins]))).astype(np.float32)
    bias = (1.0 + sig).astype(np.float32)                      # (B,N,h)

    biasT = np.repeat(bias.transpose(0, 2, 1) * scale, hd, axis=1)  # (B,C,N)
    biasT = np.ascontiguousarray(biasT, dtype=np.float32)
    import ml_dtypes
    xT = np.ascontiguousarray(
        x.transpose(0, 2, 1).astype(ml_dtypes.bfloat16))
    maskT = mask.transpose(0, 2, 1)                                # [nW,k,q]
    emT = np.ascontiguousarray(np.exp(maskT).astype(ml_dtypes.bfloat16))
    mpT = np.ascontiguousarray(
        np.rint(np.float64(AA) * maskT + (127 * 128 + MAGIC_C))
        .astype(np.int16))

    Wk = np.ascontiguousarray(Wkv[:, :C_].astype(ml_dtypes.bfloat16))
    Wv = np.ascontiguousarray(Wkv[:, C_:].astype(ml_dtypes.bfloat16))
    Wq16 = np.ascontiguousarray(Wq.astype(ml_dtypes.bfloat16))
    # Wv column order per head: [V cols][ones col skipped] - device handles
    bq2 = np.ascontiguousarray(bq.reshape(2, 128).T, dtype=np.float32)
    bk2 = np.ascontiguousarray(bkv[:C_].reshape(2, 128).T, dtype=np.float32)
    bv_rep = np.ascontiguousarray(
        np.broadcast_to(bkv[C_:], (128, C_)), dtype=np.float32)
    bp_rep = np.ascontiguousarray(
        np.broadcast_to(bp, (128, C_)), dtype=np.float32)
    # Wp permuted per otn strip: strip hp rows = [head 2hp (32), zero,
    # 31 zeros, head 2hp+1 (32), zero, 31 zeros]
    wp_perm = np.zeros((4, 128, C_), np.float32)
    for hp in range(4):
        wp_perm[hp, 0:32] = Wp[(2 * hp) * 32:(2 * hp + 1) * 32]
        wp_perm[hp, 64:96] = Wp[(2 * hp + 1) * 32:(2 * hp + 2) * 32]
        wp_perm[0, 32] = bp
    wp_perm = np.ascontiguousarray(wp_perm.astype(ml_dtypes.bfloat16))
    return xT, biasT, emT, mpT, Wq16, Wk, Wv, bq2, bk2, bv_rep, bp_rep, wp_perm


def _ensure_ntff_hook():
    import types
    try:
        from antenv import axon_hooks  # noqa: F401
        return
    except ImportError:
        pass
    import antenv
    mod = types.ModuleType("antenv.axon_hooks")
    _h = {"hook": None}
    mod.get_axon_ntff_profile_hook = lambda: _h["hook"]
    mod.set_axon_ntff_profile_hook = lambda hook: _h.__setitem__("hook", hook)
    sys.modules["antenv.axon_hooks"] = mod
    antenv.axon_hooks = mod
    try:
        sys.path.insert(0, "/root/.axon_site/trn_agent_boot")
        import trn_boot
        hook = trn_boot._ntff_profile_via_ctypes("/opt/axon/libaxon_pjrt.so")
        if hook is not None:
            mod.set_axon_ntff_profile_hook(hook)
    except Exception as e:
        print("ntff hook setup failed:", repr(e))


def kernel(x, mask, affine_matrix, Wq, bq, Wkv, bkv, Wp, bp,
           angle_table, H, W, _profile=False):
    if _profile:
        _ensure_ntff_hook()
    x = np.asarray(x, np.float32)
    mask = np.asarray(mask, np.float32)
    affine_matrix = np.asarray(affine_matrix, np.float32)
    Wq = np.asarray(Wq, np.float32); bq = np.asarray(bq, np.float32)
    Wkv = np.asarray(Wkv, np.float32); bkv = np.asarray(bkv, np.float32)
    Wp = np.asarray(Wp, np.float32); bp = np.asarray(bp, np.float32)
    angle_table = np.asarray(angle_table, np.float32)

    (xT, biasT, emT, mpT, Wq16, Wk, Wv, bq2, bk2, bv_rep, bp_rep,
     wp_perm) = _host_prep(x, mask, affine_matrix, Wq, bq, Wkv, bkv, Wp, bp,
                           angle_table, H, W)

    if "nc" not in _CACHE:
        _CACHE["nc"] = build_kernel()
    nc = _CACHE["nc"]

    import ml_dtypes
    sel = np.zeros((2, 128), ml_dtypes.bfloat16)
    sel[0, 0:64] = 1.0
    sel[1, 64:128] = 1.0

    in_maps = []
    for m in range(N_CORES):
        bs = [m, m + N_CORES]          # same mask: b % 4 == m % 4
        j = m % NW
        in_maps.append({
            "sel": sel,
            "xT": np.ascontiguousarray(xT[bs]),
            "biasT": np.ascontiguousarray(biasT[bs]),
            "em": emT[j], "mp": mpT[j],
            "wq": Wq16, "wk": Wk, "wv": Wv, "wp": wp_perm,
            "bq": bq2, "bk": bk2, "bv": bv_rep, "bp": bp_rep,
        })

    res = run_bass_kernel_spmd(nc, in_maps, core_ids=list(range(N_CORES)),
                               trace=_profile)
    out = np.empty((B, N, C), np.float32)
    for m in range(N_CORES):
        y = res.results[m]["y"]
        out[m] = y[0]
        out[m + N_CORES] = y[1]
    if _profile:
        return out, res
    return out


if __name__ == "__main__":
    import reference
    inputs = reference.setup_inputs()
    out = kernel(**{k: (np.asarray(v) if hasattr(v, "shape") else v)
                    for k, v in inputs.items()})
    ref = np.asarray(reference.reference(**inputs))
    err = np.abs(out - ref)
    print("max abs err:", err.max(),
          "absmax-rel:", err.max() / np.abs(ref).max())



# revision 4
# speedup vs baseline: 1.1047x; 1.1047x over previous
"""Trainium2 Bass kernel for nn_Attention_10797547782838.

Windowed multi-head attention with per-query angle bias:
  q = (x@Wq+bq) reshaped to heads; k,v = x@Wkv+bkv
  attn = softmax((q*scale)@k^T * anglebias(q) + mask[b%4]); out = attn@v@Wp.

Sharding: batch data-parallel over 8 cores; core m gets batches {m, m+8},
which share the same window mask (b%4 == m%4).

Device algorithm (per batch):
  - S^T layout [keys, queries]; angle bias and scale folded into q on host.
  - k-projection bias is dropped entirely: it contributes a per-query
    constant to the logits, which softmax cancels.  v-projection bias is
    folded into the output-projection bias on host (softmax weights sum
    to 1, so out += bv exactly).
  - exp(S+M) computed two ways to balance engines (chunk-assigned):
      A: ACT exp(S) -> bf16, then multiply by em=exp(M) on DVE or GPSIMD
      B: DVE scalar_tensor_tensor Schraudolph: int16(S*a + M') bitcast bf16
    where M' = rint(a*M + 16256 + c) is an int16 host-precomputed tile, so
    the Schraudolph path gets the mask add for free.  Both A-multiplies and
    the B op work on full [128, 2, 512] chunks with the 512-wide mask
    slice broadcast (stride-0) over the two head halves.
  - PV matmul uses [V_h | 1] 33-wide stationaries: partition 32 of each
    output strip accumulates the softmax denominator.
  - Denominator rows are copied out of PSUM, inverted with
    reciprocal_approx_fast, broadcast across partitions with GPSIMD
    partition_broadcast (no PE involvement), and applied to O^T on DVE
    before the output projection.
  - Output projection uses per-strip Wp with zeroed junk rows so the
    33-partition strips need no compaction.
"""
import math
import sys

import numpy as np

sys.path.insert(0, "/opt/trn_rl_repo")

import concourse.bass as bass  # noqa: E402
import concourse.bacc as bacc  # noqa: E402
import concourse.tile as tile  # noqa: E402
from concourse import mybir  # noqa: E402
from concourse.bass_utils import run_bass_kernel_spmd  # noqa: E402

F32 = mybir.dt.float32
F32R = mybir.dt.float32r
BF16 = mybir.dt.bfloat16
I16 = mybir.dt.int16
AF = mybir.ActivationFunctionType
ALU = mybir.AluOpType

B, N, C = 16, 1024, 256
HEADS, HD = 8, 32
NW = 4
N_CORES = 8
BPC = B // N_CORES  # batches per core

AA = 184.6650390625        # 128 * log2(e)
MAGIC_C = -11.0            # Schraudolph rounding correction

# per-batch chunk counts for exp paths (64 chunks of [128,1024] per batch)
# A: ACT exp + DVE mask-mult; G: ACT exp + GPSIMD mask-mult; B: DVE
# Schraudolph scalar_tensor_tensor (GPSIMD cannot read PSUM)
N_A, N_G, N_B = 19, 24, 21
LAG = 12

_CACHE = {}


def r32(ap):
    return ap.bitcast(F32R)


def bc2(a, n=2):
    # insert a stride-0 broadcast dim after the partition dim: [p, m] -> [p, n, m]
    return bass.AP(a.tensor, a.offset, [a.ap[0], [0, n]] + list(a.ap[1:]))


def make_pattern(na, ng, nb):
    tgt = {"A": na, "G": ng, "B": nb}
    cnt = {"A": 0, "G": 0, "B": 0}
    pat = []
    for _ in range(na + ng + nb):
        best, bscore = None, None
        for p in ("A", "G", "B"):
            if tgt[p] == 0 or cnt[p] >= tgt[p]:
                continue
            score = (cnt[p] + 1.0) / tgt[p]
            if bscore is None or score < bscore:
                best, bscore = p, score
        cnt[best] += 1
        pat.append(best)
    return pat


PATTERN = make_pattern(N_A, N_G, N_B)


def build_kernel():
    from contextlib import ExitStack
    nc = bacc.Bacc("TRN2", target_bir_lowering=False, debug=False,
                   num_devices=N_CORES)

    d_xT = nc.dram_tensor("xT", [BPC, C, N], BF16, kind="ExternalInput").ap()
    d_biasT = nc.dram_tensor("biasT", [BPC, C, N], BF16, kind="ExternalInput").ap()
    d_em = nc.dram_tensor("em", [N, N], BF16, kind="ExternalInput").ap()
    d_mp = nc.dram_tensor("mp", [N, N], I16, kind="ExternalInput").ap()
    d_wq = nc.dram_tensor("wq", [C, C], BF16, kind="ExternalInput").ap()
    d_wk = nc.dram_tensor("wk", [C, C], BF16, kind="ExternalInput").ap()
    d_wv = nc.dram_tensor("wv", [C, C], BF16, kind="ExternalInput").ap()
    d_wp = nc.dram_tensor("wp", [4, 128, C], BF16, kind="ExternalInput").ap()
    d_bq = nc.dram_tensor("bq", [128, 2], F32, kind="ExternalInput").ap()
    d_sel = nc.dram_tensor("sel", [2, 128], BF16, kind="ExternalInput").ap()
    d_y = nc.dram_tensor("y", [BPC, N, C], F32, kind="ExternalOutput").ap()

    with tile.TileContext(nc) as tc:
        with ExitStack() as ctx, nc.allow_low_precision(reason="bf16 matmul inputs; fp32 PSUM accumulation; approx softmax exp within tolerance"):
            kernel_body(ctx, tc, d_xT, d_biasT, d_em, d_mp, d_wq, d_wk, d_wv,
                        d_wp, d_bq, d_sel, d_y)
    nc.compile()
    return nc


def kernel_body(ctx, tc, d_xT, d_biasT, d_em, d_mp, d_wq, d_wk, d_wv, d_wp,
                d_bq, d_sel, d_y):
    nc = tc.nc

    consts = ctx.enter_context(tc.tile_pool(name="consts", bufs=1))
    xpool = ctx.enter_context(tc.tile_pool(name="xpool", bufs=2))
    qkv = ctx.enter_context(tc.tile_pool(name="qkv", bufs=2))
    vpool = ctx.enter_context(tc.tile_pool(name="vpool", bufs=2))
    ppool = ctx.enter_context(tc.tile_pool(name="ppool", bufs=15))
    otpool = ctx.enter_context(tc.tile_pool(name="otpool", bufs=2))
    misc = ctx.enter_context(tc.tile_pool(name="misc", bufs=2))
    ypool = ctx.enter_context(tc.tile_pool(name="ypool", bufs=3))
    psS = ctx.enter_context(tc.tile_pool(name="psS", bufs=2, space="PSUM"))
    psO = ctx.enter_context(tc.tile_pool(name="psO", bufs=1, space="PSUM"))

    # ---- weights + batch-0 inputs first so the first matmuls start早 ----
    w_sb = {}
    w_sb["wq"] = consts.tile([128, 2, C], BF16, tag="w_wq", name="w_wq")
    nc.sync.dma_start(out=w_sb["wq"][:],
                      in_=d_wq.rearrange("(c p) n -> p c n", p=128))
    bq_sb = consts.tile([128, 2], F32, tag="bq")
    nc.sync.dma_start(out=bq_sb[:], in_=d_bq[:])

    xT_sb, biasT_sb, qT_sb, kT_sb, v_sb = {}, {}, {}, {}, {}
    for b in range(BPC):
        xT_sb[b] = xpool.tile([128, 2, N], BF16, tag="xT", name=f"xT{b}")
        nc.sync.dma_start(out=xT_sb[b][:],
                          in_=d_xT[b].rearrange("(c p) n -> p c n", p=128))
        biasT_sb[b] = xpool.tile([128, 2, N], BF16, tag="biasT", name=f"bT{b}")
        nc.sync.dma_start(out=biasT_sb[b][:],
                          in_=d_biasT[b].rearrange("(c p) n -> p c n", p=128))
        if b == 0:
            w_sb["wk"] = consts.tile([128, 2, C], BF16, tag="w_wk", name="w_wk")
            nc.sync.dma_start(out=w_sb["wk"][:],
                              in_=d_wk.rearrange("(c p) n -> p c n", p=128))
            w_sb["wv"] = consts.tile([128, 2, C], BF16, tag="w_wv", name="w_wv")
            nc.sync.dma_start(out=w_sb["wv"][:],
                              in_=d_wv.rearrange("(c p) n -> p c n", p=128))

    # masks staged once per core (both batches share b%4)
    em_sb = [consts.tile([128, N], BF16, tag=f"em{kc}", name=f"em{kc}")
             for kc in range(8)]
    mp_sb = [consts.tile([128, N], I16, tag=f"mp{kc}", name=f"mp{kc}")
             for kc in range(8)]
    for kc in range(8):
        nc.sync.dma_start(out=em_sb[kc][:], in_=d_em[kc * 128:(kc + 1) * 128, :])
        nc.sync.dma_start(out=mp_sb[kc][:], in_=d_mp[kc * 128:(kc + 1) * 128, :])

    # wp comes pre-permuted per otn strip: [4 strips, 128 rows, C]
    # (strip 0 row 32 carries bp + bv@Wp so no separate bias add is needed)
    wp_sb = consts.tile([128, 4, C], BF16, tag="w_wp")
    nc.sync.dma_start(out=wp_sb[:], in_=d_wp.rearrange("s p n -> p s n"))
    # selector rows: selA -> partitions 0:64, selB -> partitions 64:128
    selA = consts.tile([1, 128], BF16, tag="selA")
    nc.sync.dma_start(out=selA[:], in_=d_sel[0:1, :])
    selB = consts.tile([1, 128], BF16, tag="selB")
    nc.sync.dma_start(out=selB[:], in_=d_sel[1:2, :])

    # ---- projections for both batches ----
    for b in range(BPC):
        qT_sb[b] = qkv.tile([128, 2, N], BF16, tag="qT", name=f"qT{b}")
        kT_sb[b] = qkv.tile([128, 2, N], BF16, tag="kT", name=f"kT{b}")
        for m in range(2):
            for qc in range(2):
                ps_q = psS.tile([128, 512], F32, tag="s")
                for ci in range(2):
                    nc.tensor.matmul(
                        ps_q[:],
                        w_sb["wq"][:, ci, m * 128:(m + 1) * 128],
                        xT_sb[b][:, ci, qc * 512:(qc + 1) * 512],
                        start=(ci == 0), stop=(ci == 1))
                nc.vector.scalar_tensor_tensor(
                    out=qT_sb[b][:, m, qc * 512:(qc + 1) * 512], in0=ps_q[:],
                    scalar=bq_sb[:, m:m + 1],
                    in1=biasT_sb[b][:, m, qc * 512:(qc + 1) * 512],
                    op0=ALU.add, op1=ALU.mult)
                ps_k = psS.tile([128, 512], F32, tag="s")
                for ci in range(2):
                    nc.tensor.matmul(
                        ps_k[:],
                        w_sb["wk"][:, ci, m * 128:(m + 1) * 128],
                        xT_sb[b][:, ci, qc * 512:(qc + 1) * 512],
                        start=(ci == 0), stop=(ci == 1))
                # k bias dropped: softmax cancels per-query logit offsets
                keng = nc.scalar if (m + qc) % 2 == 0 else nc.vector
                if keng is nc.scalar:
                    keng.copy(
                        out=kT_sb[b][:, m, qc * 512:(qc + 1) * 512], in_=ps_k[:])
                else:
                    keng.tensor_scalar_add(
                        out=kT_sb[b][:, m, qc * 512:(qc + 1) * 512],
                        in0=ps_k[:], scalar1=0.0)

        v_sb[b] = []
        for t8 in range(8):
            vt = vpool.tile([128, 8, 33], BF16, tag=f"v{t8}", name=f"v{b}_{t8}")
            ps_v = psS.tile([128, C], F32, tag="s")
            for ci in range(2):
                nc.tensor.matmul(
                    ps_v[:],
                    xT_sb[b][:, ci, t8 * 128:(t8 + 1) * 128],
                    w_sb["wv"][:, ci, :],
                    start=(ci == 0), stop=(ci == 1))
            # bv folded into wp bias row on host
            if t8 % 2 == 0:
                nc.scalar.copy(out=vt[:, :, 0:32], in_=ps_v[:])
            else:
                nc.vector.tensor_scalar_add(out=vt[:, :, 0:32], in0=ps_v[:],
                                            scalar1=0.0)
            nc.gpsimd.memset(vt[:, :, 32:33], 1.0)
            v_sb[b].append(vt)

    # ---- attention: batches interleaved; PV matmuls and tail pieces are
    # ---- emitted with a lag so the in-order PE queue never head-of-line
    # ---- blocks on the exp/mask path of the current chunk
    from collections import deque
    pending = deque()

    def step(fn):
        pending.append(fn)
        while len(pending) > LAG:
            pending.popleft()()

    chunk_i = {0: 0, 1: 0}
    for qc in range(2):
        for b in range(BPC):
            pot = psO.tile([128, 4, 512], F32, tag="po")
            for kc in range(8):
                for hp in range(4):  # head pair (2hp, 2hp+1)
                    path = PATTERN[chunk_i[b]]
                    chunk_i[b] += 1
                    ps_s = psS.tile([128, N], F32, tag="s")
                    for hh in range(2):
                        h = 2 * hp + hh
                        j = h % 4
                        nc.tensor.matmul(
                            ps_s[:, hh * 512:(hh + 1) * 512],
                            kT_sb[b][32 * j:32 * (j + 1), h // 4,
                                     kc * 128:(kc + 1) * 128],
                            qT_sb[b][32 * j:32 * (j + 1), h // 4,
                                     qc * 512:(qc + 1) * 512],
                            start=True, stop=True,
                            tile_position=(32 * j, 0))
                    emsl = em_sb[kc][:, qc * 512:(qc + 1) * 512]
                    mpsl = mp_sb[kc][:, qc * 512:(qc + 1) * 512]
                    if path in ("A", "G"):
                        p0 = ppool.tile([128, N], BF16, tag="p0")
                        pt = ppool.tile([128, N], BF16, tag="pt")
                        nc.scalar.activation(out=p0[:], in_=ps_s[:], func=AF.Exp)
                        eng = nc.vector if path == "A" else nc.gpsimd
                        eng.tensor_tensor(
                            out=pt[:].rearrange("p (h q) -> p h q", h=2),
                            in0=p0[:].rearrange("p (h q) -> p h q", h=2),
                            in1=bc2(emsl), op=ALU.mult)
                        pt_bf = pt
                    else:
                        pti = ppool.tile([128, N], I16, tag="pt")
                        nc.vector.scalar_tensor_tensor(
                            out=pti[:].rearrange("p (h q) -> p h q", h=2),
                            in0=ps_s[:].rearrange("p (h q) -> p h q", h=2),
                            scalar=AA, in1=bc2(mpsl),
                            op0=ALU.mult, op1=ALU.add)
                        pt_bf = pti.bitcast(BF16)

                    def po_mms(pot=pot, b=b, kc=kc, hp=hp, pt_bf=pt_bf):
                        for hh in range(2):
                            h = 2 * hp + hh
                            nc.tensor.matmul(
                                pot[64 * hh:64 * hh + 33, hp, :],
                                v_sb[b][kc][:, h, :],
                                pt_bf[:, hh * 512:(hh + 1) * 512],
                                start=(kc == 0), stop=(kc == 7),
                                tile_position=(0, 64 * hh))
                    step(po_mms)

            def tail(pot=pot, b=b, qc=qc):
                # gather the denominator rows (partitions 32, 96) as bf16
                rs0 = misc.tile([1, 4, 512], BF16, tag="rs0")
                nc.scalar.copy(out=rs0[:], in_=pot[32:33, :, :])
                rs1 = misc.tile([1, 4, 512], BF16, tag="rs1")
                nc.vector.tensor_scalar_add(out=rs1[:],
                                            in0=pot[96:97, :, :], scalar1=0.0)
                # broadcast across partitions via 1-row selector matmuls
                # (rows 0-63 head 2hp, rows 64-127 head 2hp+1), then invert
                # straight out of PSUM and apply to the O^T strip
                otn = []
                for hp in range(4):
                    ps_rb = psS.tile([128, 512], F32, tag="s")
                    nc.tensor.matmul(ps_rb[:], selA[:], rs0[:, hp, :],
                                     start=True, stop=False)
                    nc.tensor.matmul(ps_rb[:], selB[:], rs1[:, hp, :],
                                     start=False, stop=True)
                    rbs = misc.tile([128, 512], F32, tag="rbs")
                    nc.vector.reciprocal_approx_fast(out=rbs[:], in_=ps_rb[:])
                    ot = otpool.tile([128, 512], BF16, tag=f"otn{hp}",
                                     name=f"ot{hp}")
                    nc.vector.tensor_tensor(
                        out=ot[:], in0=pot[:, hp, :], in1=rbs[:],
                        op=ALU.mult)
                    otn.append(ot)
                for qt in range(4):
                    ps_y = psS.tile([128, C], F32, tag="s")
                    for hp in range(4):
                        nc.tensor.matmul(
                            ps_y[:],
                            otn[hp][:, qt * 128:(qt + 1) * 128],
                            wp_sb[:, hp, :],
                            start=(hp == 0), stop=(hp == 3))
                    y_sb = ypool.tile([128, C], F32, tag="y")
                    nc.scalar.copy(out=y_sb[:], in_=ps_y[:])
                    qoff = qc * 512 + qt * 128
                    nc.sync.dma_start(out=d_y[b, qoff:qoff + 128, :],
                                      in_=y_sb[:])
            step(tail)

    while pending:
        pending.popleft()()


def _host_prep(x, mask, affine_matrix, Wq, bq, Wkv, bkv, Wp, bp,
               angle_table, H, W):
    B_, N_, C_ = x.shape
    heads = angle_table.shape[1]
    hd = C_ // heads
    scale = np.float32(hd ** -0.5)
    H = int(H); W = int(W)

    gy, gx = np.meshgrid(np.arange(H, dtype=np.float32),
                         np.arange(W, dtype=np.float32), indexing="ij")
    coords = np.stack([gx.reshape(-1), gy.reshape(-1)], -1).astype(np.float32)
    center = np.array([W / 2.0, H / 2.0], np.float32)
    ego = np.einsum("bij,j->bi", affine_matrix[:, :2, :2], center) \
        + affine_matrix[:, :2, 2]
    rel = coords[None, :, :] - ego[:, None, :]
    ang = np.arctan2(rel[..., 1], rel[..., 0]).astype(np.float32)
    bins = (((ang + np.float32(math.pi)) / np.float32(2.0 * math.pi))
            * (angle_table.shape[0] - 1)).astype(np.int32)
    sig = (1.0 / (1.0 + np.exp(-angle_table[bins]))).astype(np.float32)
    bias = (1.0 + sig).astype(np.float32)                      # (B,N,h)

    import ml_dtypes
    biasT = np.repeat(bias.transpose(0, 2, 1) * scale, hd, axis=1)  # (B,C,N)
    biasT = np.ascontiguousarray(biasT).astype(ml_dtypes.bfloat16)
    xT = np.ascontiguousarray(
        x.transpose(0, 2, 1).astype(ml_dtypes.bfloat16))
    maskT = mask.transpose(0, 2, 1)                                # [nW,k,q]
    emT = np.ascontiguousarray(np.exp(maskT).astype(ml_dtypes.bfloat16))
    mpT = np.ascontiguousarray(
        np.rint(np.float64(AA) * maskT + (127 * 128 + MAGIC_C))
        .astype(np.int16))

    Wk = np.ascontiguousarray(Wkv[:, :C_].astype(ml_dtypes.bfloat16))
    Wv = np.ascontiguousarray(Wkv[:, C_:].astype(ml_dtypes.bfloat16))
    Wq16 = np.ascontiguousarray(Wq.astype(ml_dtypes.bfloat16))
    bq2 = np.ascontiguousarray(bq.reshape(2, 128).T, dtype=np.float32)
    # Wp permuted per otn strip: strip hp rows = [head 2hp (32), bias row,
    # 31 zeros, head 2hp+1 (32), zero, 31 zeros]; the bias row carries
    # bp + bv@Wp (v bias folded through the output projection)
    bv = bkv[C_:]
    bp_eff = bp + bv @ Wp
    wp_perm = np.zeros((4, 128, C_), np.float32)
    for hp in range(4):
        wp_perm[hp, 0:32] = Wp[(2 * hp) * 32:(2 * hp + 1) * 32]
        wp_perm[hp, 64:96] = Wp[(2 * hp + 1) * 32:(2 * hp + 2) * 32]
    wp_perm[0, 32] = bp_eff
    wp_perm = np.ascontiguousarray(wp_perm.astype(ml_dtypes.bfloat16))
    return xT, biasT, emT, mpT, Wq16, Wk, Wv, bq2, wp_perm


def _ensure_ntff_hook():
    import types
    try:
        from antenv import axon_hooks  # noqa: F401
        return
    except ImportError:
        pass
    import antenv
    mod = types.ModuleType("antenv.axon_hooks")
    _h = {"hook": None}
    mod.get_axon_ntff_profile_hook = lambda: _h["hook"]
    mod.set_axon_ntff_profile_hook = lambda hook: _h.__setitem__("hook", hook)
    sys.modules["antenv.axon_hooks"] = mod
    antenv.axon_hooks = mod
    try:
        sys.path.insert(0, "/root/.axon_site/trn_agent_boot")
        import trn_boot
        hook = trn_boot._ntff_profile_via_ctypes("/opt/axon/libaxon_pjrt.so")
        if hook is not None:
            mod.set_axon_ntff_profile_hook(hook)
    except Exception as e:
        print("ntff hook setup failed:", repr(e))


def kernel(x, mask, affine_matrix, Wq, bq, Wkv, bkv, Wp, bp,
           angle_table, H, W, _profile=False):
    if _profile:
        _ensure_ntff_hook()
    x = np.asarray(x, np.float32)
    mask = np.asarray(mask, np.float32)
    affine_matrix = np.asarray(affine_matrix, np.float32)
    Wq = np.asarray(Wq, np.float32); bq = np.asarray(bq, np.float32)
    Wkv = np.asarray(Wkv, np.float32); bkv = np.asarray(bkv, np.float32)
    Wp = np.asarray(Wp, np.float32); bp = np.asarray(bp, np.float32)
    angle_table = np.asarray(angle_table, np.float32)

    (xT, biasT, emT, mpT, Wq16, Wk, Wv, bq2,
     wp_perm) = _host_prep(x, mask, affine_matrix, Wq, bq, Wkv, bkv, Wp, bp,
                           angle_table, H, W)

    if "nc" not in _CACHE:
        _CACHE["nc"] = build_kernel()
    nc = _CACHE["nc"]

    import ml_dtypes
    sel = np.zeros((2, 128), ml_dtypes.bfloat16)
    sel[0, 0:64] = 1.0
    sel[1, 64:128] = 1.0

    in_maps = []
    for m in range(N_CORES):
        bs = [m, m + N_CORES]          # same mask: b % 4 == m % 4
        j = m % NW
        in_maps.append({
            "xT": np.ascontiguousarray(xT[bs]),
            "biasT": np.ascontiguousarray(biasT[bs]),
            "em": emT[j], "mp": mpT[j],
            "wq": Wq16, "wk": Wk, "wv": Wv, "wp": wp_perm,
            "bq": bq2, "sel": sel,
        })

    res = run_bass_kernel_spmd(nc, in_maps, core_ids=list(range(N_CORES)),
                               trace=_profile)
    out = np.empty((B, N, C), np.float32)
    for m in range(N_CORES):
        y = res.results[m]["y"]
        out[m] = y[0]
        out[m + N_CORES] = y[1]
    if _profile:
        return out, res
    return out


if __name__ == "__main__":
    import reference
    inputs = reference.setup_inputs()
    out = kernel(**{k: (np.asarray(v) if hasattr(v, "shape") else v)
                    for k, v in inputs.items()})
    ref = np.asarray(reference.reference(**inputs))
    err = np.abs(out - ref)
    print("max abs err:", err.max(),
          "absmax-rel:", err.max() / np.abs(ref).max())


# revision 7
# speedup vs baseline: 1.1340x; 1.0265x over previous
"""Trainium2 Bass kernel for nn_Attention_10797547782838.

Windowed multi-head attention with per-query angle bias:
  q = (x@Wq+bq) reshaped to heads; k,v = x@Wkv+bkv
  attn = softmax((q*scale)@k^T * anglebias(q) + mask[b%4]); out = attn@v@Wp.

Sharding: batch data-parallel over 8 cores; core m gets batches {m, m+8},
which share the same window mask (b%4 == m%4).

Device algorithm (per batch):
  - S^T layout [keys, queries]; angle bias and scale folded into q on host.
  - k-projection bias is dropped entirely: it contributes a per-query
    constant to the logits, which softmax cancels.  v-projection bias is
    folded into the output-projection bias on host (softmax weights sum
    to 1, so out += bv exactly).
  - exp(S+M) computed two ways to balance engines (chunk-assigned):
      A: ACT exp(S) -> bf16, then multiply by em=exp(M) on DVE or GPSIMD
      B: DVE scalar_tensor_tensor Schraudolph: int16(S*a + M') bitcast bf16
    where M' = rint(a*M + 16256 + c) is an int16 host-precomputed tile, so
    the Schraudolph path gets the mask add for free.  Both A-multiplies and
    the B op work on full [128, 2, 512] chunks with the 512-wide mask
    slice broadcast (stride-0) over the two head halves.
  - PV matmul uses [V_h | 1] 33-wide stationaries: partition 32 of each
    output strip accumulates the softmax denominator.
  - Denominator rows are copied out of PSUM, inverted with
    reciprocal_approx_fast, broadcast across partitions with GPSIMD
    partition_broadcast (no PE involvement), and applied to O^T on DVE
    before the output projection.
  - Output projection uses per-strip Wp with zeroed junk rows so the
    33-partition strips need no compaction.
"""
import math
import sys

import numpy as np

sys.path.insert(0, "/opt/trn_rl_repo")

import concourse.bass as bass  # noqa: E402
import concourse.bacc as bacc  # noqa: E402
import concourse.tile as tile  # noqa: E402
from concourse import mybir  # noqa: E402
from concourse.bass_utils import run_bass_kernel_spmd  # noqa: E402

F32 = mybir.dt.float32
F32R = mybir.dt.float32r
BF16 = mybir.dt.bfloat16
I16 = mybir.dt.int16
AF = mybir.ActivationFunctionType
ALU = mybir.AluOpType

B, N, C = 16, 1024, 256
HEADS, HD = 8, 32
NW = 4
N_CORES = 8
BPC = B // N_CORES  # batches per core

AA = 184.6650390625        # 128 * log2(e)
MAGIC_C = -11.0            # Schraudolph rounding correction

# per-batch chunk counts for exp paths (64 chunks of [128,1024] per batch)
# A: ACT exp + DVE mask-mult; G: ACT exp + GPSIMD mask-mult; B: DVE
# Schraudolph scalar_tensor_tensor (GPSIMD cannot read PSUM)
N_A, N_G, N_B = 19, 24, 21
LAG = 16

_CACHE = {}


def r32(ap):
    return ap.bitcast(F32R)


def bc2(a, n=2):
    # insert a stride-0 broadcast dim after the partition dim: [p, m] -> [p, n, m]
    return bass.AP(a.tensor, a.offset, [a.ap[0], [0, n]] + list(a.ap[1:]))


def make_pattern(na, ng, nb):
    tgt = {"A": na, "G": ng, "B": nb}
    cnt = {"A": 0, "G": 0, "B": 0}
    pat = []
    for _ in range(na + ng + nb):
        best, bscore = None, None
        for p in ("A", "G", "B"):
            if tgt[p] == 0 or cnt[p] >= tgt[p]:
                continue
            score = (cnt[p] + 1.0) / tgt[p]
            if bscore is None or score < bscore:
                best, bscore = p, score
        cnt[best] += 1
        pat.append(best)
    return pat


PATTERN = make_pattern(N_A, N_G, N_B)


def build_kernel():
    from contextlib import ExitStack
    nc = bacc.Bacc("TRN2", target_bir_lowering=False, debug=False,
                   num_devices=N_CORES)

    d_xT = nc.dram_tensor("xT", [BPC, C, N], BF16, kind="ExternalInput").ap()
    d_biasT = nc.dram_tensor("biasT", [BPC, C, N], BF16, kind="ExternalInput").ap()
    d_em = nc.dram_tensor("em", [N, N], BF16, kind="ExternalInput").ap()
    d_mp = nc.dram_tensor("mp", [N, N], I16, kind="ExternalInput").ap()
    d_wq = nc.dram_tensor("wq", [C, C], BF16, kind="ExternalInput").ap()
    d_wk = nc.dram_tensor("wk", [C, C], BF16, kind="ExternalInput").ap()
    d_wv = nc.dram_tensor("wv", [C, C], BF16, kind="ExternalInput").ap()
    d_wp = nc.dram_tensor("wp", [4, 128, C], BF16, kind="ExternalInput").ap()
    d_bq = nc.dram_tensor("bq", [128, 2], F32, kind="ExternalInput").ap()
    d_sel = nc.dram_tensor("sel", [2, 128], BF16, kind="ExternalInput").ap()
    d_y = nc.dram_tensor("y", [BPC, N, C], F32, kind="ExternalOutput").ap()

    with tile.TileContext(nc) as tc:
        with ExitStack() as ctx, nc.allow_low_precision(reason="bf16 matmul inputs; fp32 PSUM accumulation; approx softmax exp within tolerance"):
            kernel_body(ctx, tc, d_xT, d_biasT, d_em, d_mp, d_wq, d_wk, d_wv,
                        d_wp, d_bq, d_sel, d_y)
    nc.compile()
    return nc


def kernel_body(ctx, tc, d_xT, d_biasT, d_em, d_mp, d_wq, d_wk, d_wv, d_wp,
                d_bq, d_sel, d_y):
    nc = tc.nc

    consts = ctx.enter_context(tc.tile_pool(name="consts", bufs=1))
    xpool = ctx.enter_context(tc.tile_pool(name="xpool", bufs=2))
    qkv = ctx.enter_context(tc.tile_pool(name="qkv", bufs=2))
    vpool = ctx.enter_context(tc.tile_pool(name="vpool", bufs=2))
    ppool = ctx.enter_context(tc.tile_pool(name="ppool", bufs=19))
    otpool = ctx.enter_context(tc.tile_pool(name="otpool", bufs=2))
    misc = ctx.enter_context(tc.tile_pool(name="misc", bufs=2))
    ypool = ctx.enter_context(tc.tile_pool(name="ypool", bufs=3))
    psS = ctx.enter_context(tc.tile_pool(name="psS", bufs=2, space="PSUM"))
    psO = ctx.enter_context(tc.tile_pool(name="psO", bufs=1, space="PSUM"))

    # ---- weights + batch-0 inputs first so the first matmuls start早 ----
    w_sb = {}
    w_sb["wq"] = consts.tile([128, 2, C], BF16, tag="w_wq", name="w_wq")
    nc.sync.dma_start(out=w_sb["wq"][:],
                      in_=d_wq.rearrange("(c p) n -> p c n", p=128))
    bq_sb = consts.tile([128, 2], F32, tag="bq")
    nc.sync.dma_start(out=bq_sb[:], in_=d_bq[:])

    xT_sb, biasT_sb, qT_sb, kT_sb, v_sb = {}, {}, {}, {}, {}
    for b in range(BPC):
        xT_sb[b] = xpool.tile([128, 2, N], BF16, tag="xT", name=f"xT{b}")
        nc.sync.dma_start(out=xT_sb[b][:],
                          in_=d_xT[b].rearrange("(c p) n -> p c n", p=128))
        biasT_sb[b] = xpool.tile([128, 2, N], BF16, tag="biasT", name=f"bT{b}")
        nc.sync.dma_start(out=biasT_sb[b][:],
                          in_=d_biasT[b].rearrange("(c p) n -> p c n", p=128))
        if b == 0:
            w_sb["wk"] = consts.tile([128, 2, C], BF16, tag="w_wk", name="w_wk")
            nc.sync.dma_start(out=w_sb["wk"][:],
                              in_=d_wk.rearrange("(c p) n -> p c n", p=128))
            w_sb["wv"] = consts.tile([128, 2, C], BF16, tag="w_wv", name="w_wv")
            nc.sync.dma_start(out=w_sb["wv"][:],
                              in_=d_wv.rearrange("(c p) n -> p c n", p=128))

    # masks staged once per core (both batches share b%4)
    em_sb = [consts.tile([128, N], BF16, tag=f"em{kc}", name=f"em{kc}")
             for kc in range(8)]
    mp_sb = [consts.tile([128, N], I16, tag=f"mp{kc}", name=f"mp{kc}")
             for kc in range(8)]
    for kc in range(8):
        nc.sync.dma_start(out=em_sb[kc][:], in_=d_em[kc * 128:(kc + 1) * 128, :])
        nc.sync.dma_start(out=mp_sb[kc][:], in_=d_mp[kc * 128:(kc + 1) * 128, :])

    # wp comes pre-permuted per otn strip: [4 strips, 128 rows, C]
    # (strip 0 row 32 carries bp + bv@Wp so no separate bias add is needed)
    wp_sb = consts.tile([128, 4, C], BF16, tag="w_wp")
    nc.sync.dma_start(out=wp_sb[:], in_=d_wp.rearrange("s p n -> p s n"))
    # selector rows: selA -> partitions 0:64, selB -> partitions 64:128
    selA = consts.tile([1, 128], BF16, tag="selA")
    nc.sync.dma_start(out=selA[:], in_=d_sel[0:1, :])
    selB = consts.tile([1, 128], BF16, tag="selB")
    nc.sync.dma_start(out=selB[:], in_=d_sel[1:2, :])

    # ---- projections for both batches ----
    for b in range(BPC):
        qT_sb[b] = qkv.tile([128, 2, N], BF16, tag="qT", name=f"qT{b}")
        kT_sb[b] = qkv.tile([128, 2, N], BF16, tag="kT", name=f"kT{b}")
        for m in range(2):
            for qc in range(2):
                ps_q = psS.tile([128, 512], F32, tag="s")
                for ci in range(2):
                    nc.tensor.matmul(
                        ps_q[:],
                        w_sb["wq"][:, ci, m * 128:(m + 1) * 128],
                        xT_sb[b][:, ci, qc * 512:(qc + 1) * 512],
                        start=(ci == 0), stop=(ci == 1))
                nc.vector.scalar_tensor_tensor(
                    out=qT_sb[b][:, m, qc * 512:(qc + 1) * 512], in0=ps_q[:],
                    scalar=bq_sb[:, m:m + 1],
                    in1=biasT_sb[b][:, m, qc * 512:(qc + 1) * 512],
                    op0=ALU.add, op1=ALU.mult)
                ps_k = psS.tile([128, 512], F32, tag="s")
                for ci in range(2):
                    nc.tensor.matmul(
                        ps_k[:],
                        w_sb["wk"][:, ci, m * 128:(m + 1) * 128],
                        xT_sb[b][:, ci, qc * 512:(qc + 1) * 512],
                        start=(ci == 0), stop=(ci == 1))
                # k bias dropped: softmax cancels per-query logit offsets
                keng = nc.scalar if (m + qc) % 2 == 0 else nc.vector
                if keng is nc.scalar:
                    keng.copy(
                        out=kT_sb[b][:, m, qc * 512:(qc + 1) * 512], in_=ps_k[:])
                else:
                    keng.tensor_scalar_add(
                        out=kT_sb[b][:, m, qc * 512:(qc + 1) * 512],
                        in0=ps_k[:], scalar1=0.0)

        v_sb[b] = []
        for t8 in range(8):
            vt = vpool.tile([128, 8, 33], BF16, tag=f"v{t8}", name=f"v{b}_{t8}")
            ps_v = psS.tile([128, C], F32, tag="s")
            for ci in range(2):
                nc.tensor.matmul(
                    ps_v[:],
                    xT_sb[b][:, ci, t8 * 128:(t8 + 1) * 128],
                    w_sb["wv"][:, ci, :],
                    start=(ci == 0), stop=(ci == 1))
            # bv folded into wp bias row on host
            if t8 % 2 == 0:
                nc.scalar.copy(out=vt[:, :, 0:32], in_=ps_v[:])
            else:
                nc.vector.tensor_scalar_add(out=vt[:, :, 0:32], in0=ps_v[:],
                                            scalar1=0.0)
            nc.gpsimd.memset(vt[:, :, 32:33], 1.0)
            v_sb[b].append(vt)

    # ---- attention: batches interleaved; PV matmuls and tail pieces are
    # ---- emitted with a lag so the in-order PE queue never head-of-line
    # ---- blocks on the exp/mask path of the current chunk
    from collections import deque
    pending = deque()
    tail_state = {}

    def step(fn):
        pending.append(fn)
        while len(pending) > LAG:
            pending.popleft()()

    chunk_i = {0: 0, 1: 0}
    for qc in range(2):
        for b in range(BPC):
            pot = psO.tile([128, 4, 512], F32, tag="po")
            for kc in range(8):
                for hp in range(4):  # head pair (2hp, 2hp+1)
                    path = PATTERN[chunk_i[b]]
                    chunk_i[b] += 1
                    ps_s = psS.tile([128, N], F32, tag="s")
                    for hh in range(2):
                        h = 2 * hp + hh
                        j = h % 4
                        nc.tensor.matmul(
                            ps_s[:, hh * 512:(hh + 1) * 512],
                            kT_sb[b][32 * j:32 * (j + 1), h // 4,
                                     kc * 128:(kc + 1) * 128],
                            qT_sb[b][32 * j:32 * (j + 1), h // 4,
                                     qc * 512:(qc + 1) * 512],
                            start=True, stop=True,
                            tile_position=(32 * j, 0))
                    emsl = em_sb[kc][:, qc * 512:(qc + 1) * 512]
                    mpsl = mp_sb[kc][:, qc * 512:(qc + 1) * 512]
                    if path in ("A", "G"):
                        p0 = ppool.tile([128, N], BF16, tag="p0")
                        pt = ppool.tile([128, N], BF16, tag="pt")
                        nc.scalar.activation(out=p0[:], in_=ps_s[:], func=AF.Exp)
                        eng = nc.vector if path == "A" else nc.gpsimd
                        eng.tensor_tensor(
                            out=pt[:].rearrange("p (h q) -> p h q", h=2),
                            in0=p0[:].rearrange("p (h q) -> p h q", h=2),
                            in1=bc2(emsl), op=ALU.mult)
                        pt_bf = pt
                    else:
                        pti = ppool.tile([128, N], I16, tag="pt")
                        nc.vector.scalar_tensor_tensor(
                            out=pti[:].rearrange("p (h q) -> p h q", h=2),
                            in0=ps_s[:].rearrange("p (h q) -> p h q", h=2),
                            scalar=AA, in1=bc2(mpsl),
                            op0=ALU.mult, op1=ALU.add)
                        pt_bf = pti.bitcast(BF16)

                    def po_mms(pot=pot, b=b, kc=kc, hp=hp, pt_bf=pt_bf):
                        for hh in range(2):
                            h = 2 * hp + hh
                            nc.tensor.matmul(
                                pot[64 * hh:64 * hh + 33, hp, :],
                                v_sb[b][kc][:, h, :],
                                pt_bf[:, hh * 512:(hh + 1) * 512],
                                start=(kc == 0), stop=(kc == 7),
                                tile_position=(0, 64 * hh))
                    step(po_mms)

            otn = [None] * 4

            def tail_a(pot=pot, otn=otn):
                # gather the denominator rows (partitions 32, 96) as bf16,
                # broadcast across partitions via 1-row selector matmuls
                # (rows 0-63 head 2hp, rows 64-127 head 2hp+1), invert
                # straight out of PSUM, apply to the O^T strips (hp 0,1)
                rs0 = misc.tile([1, 4, 512], BF16, tag="rs0")
                nc.scalar.copy(out=rs0[:], in_=pot[32:33, :, :])
                rs1 = misc.tile([1, 4, 512], BF16, tag="rs1")
                nc.vector.tensor_scalar_add(out=rs1[:],
                                            in0=pot[96:97, :, :], scalar1=0.0)
                ps_rb = psS.tile([128, 2, 512], F32, tag="s")
                for j in range(2):
                    nc.tensor.matmul(ps_rb[:, j, :], selA[:], rs0[:, j, :],
                                     start=True, stop=False)
                    nc.tensor.matmul(ps_rb[:, j, :], selB[:], rs1[:, j, :],
                                     start=False, stop=True)
                rbs = misc.tile([128, 2, 512], F32, tag="rbs")
                nc.vector.reciprocal_approx_fast(out=rbs[:], in_=ps_rb[:])
                tail_state[(id(pot), 'a')] = (rs0, rs1, rbs)
                for hp in range(2):
                    ot = otpool.tile([128, 512], BF16, tag=f"otn{hp}",
                                     name=f"ot{hp}")
                    nc.vector.tensor_tensor(
                        out=ot[:], in0=pot[:, hp, :], in1=rbs[:, hp, :],
                        op=ALU.mult)
                    otn[hp] = ot

            def tail_b(pot=pot, otn=otn):
                rs0, rs1, _ = tail_state[(id(pot), 'a')]
                ps_rb = psS.tile([128, 2, 512], F32, tag="s")
                for j in range(2):
                    nc.tensor.matmul(ps_rb[:, j, :], selA[:], rs0[:, 2 + j, :],
                                     start=True, stop=False)
                    nc.tensor.matmul(ps_rb[:, j, :], selB[:], rs1[:, 2 + j, :],
                                     start=False, stop=True)
                rbs = misc.tile([128, 2, 512], F32, tag="rbs")
                nc.vector.reciprocal_approx_fast(out=rbs[:], in_=ps_rb[:])
                for hp in range(2, 4):
                    ot = otpool.tile([128, 512], BF16, tag=f"otn{hp}",
                                     name=f"ot{hp}")
                    nc.vector.tensor_tensor(
                        out=ot[:], in0=pot[:, hp, :], in1=rbs[:, hp - 2, :],
                        op=ALU.mult)
                    otn[hp] = ot

            def tail_y(qp, b=b, qc=qc, otn=otn):
                # two q-tiles of the output projection per call
                ps_y = psS.tile([128, 2, C], F32, tag="s")
                for j in range(2):
                    qt = 2 * qp + j
                    for hp in range(4):
                        nc.tensor.matmul(
                            ps_y[:, j, :],
                            otn[hp][:, qt * 128:(qt + 1) * 128],
                            wp_sb[:, hp, :],
                            start=(hp == 0), stop=(hp == 3))
                y_sb = ypool.tile([128, 2, C], F32, tag="y")
                nc.scalar.copy(out=y_sb[:], in_=ps_y[:])
                qoff = qc * 512 + qp * 256
                nc.sync.dma_start(
                    out=d_y[b, qoff:qoff + 256, :].rearrange(
                        "(a p) n -> p a n", p=128),
                    in_=y_sb[:])

            step(tail_a)
            step(tail_b)
            step(lambda f=tail_y: f(0))
            step(lambda f=tail_y: f(1))

    while pending:
        pending.popleft()()


def _host_prep(x, mask, affine_matrix, Wq, bq, Wkv, bkv, Wp, bp,
               angle_table, H, W):
    B_, N_, C_ = x.shape
    heads = angle_table.shape[1]
    hd = C_ // heads
    scale = np.float32(hd ** -0.5)
    H = int(H); W = int(W)

    gy, gx = np.meshgrid(np.arange(H, dtype=np.float32),
                         np.arange(W, dtype=np.float32), indexing="ij")
    coords = np.stack([gx.reshape(-1), gy.reshape(-1)], -1).astype(np.float32)
    center = np.array([W / 2.0, H / 2.0], np.float32)
    ego = np.einsum("bij,j->bi", affine_matrix[:, :2, :2], center) \
        + affine_matrix[:, :2, 2]
    rel = coords[None, :, :] - ego[:, None, :]
    ang = np.arctan2(rel[..., 1], rel[..., 0]).astype(np.float32)
    bins = (((ang + np.float32(math.pi)) / np.float32(2.0 * math.pi))
            * (angle_table.shape[0] - 1)).astype(np.int32)
    sig = (1.0 / (1.0 + np.exp(-angle_table[bins]))).astype(np.float32)
    bias = (1.0 + sig).astype(np.float32)                      # (B,N,h)

    import ml_dtypes
    biasT = np.repeat(bias.transpose(0, 2, 1) * scale, hd, axis=1)  # (B,C,N)
    biasT = np.ascontiguousarray(biasT).astype(ml_dtypes.bfloat16)
    xT = np.ascontiguousarray(
        x.transpose(0, 2, 1).astype(ml_dtypes.bfloat16))
    maskT = mask.transpose(0, 2, 1)                                # [nW,k,q]
    emT = np.ascontiguousarray(np.exp(maskT).astype(ml_dtypes.bfloat16))
    mpT = np.ascontiguousarray(
        np.rint(np.float64(AA) * maskT + (127 * 128 + MAGIC_C))
        .astype(np.int16))

    Wk = np.ascontiguousarray(Wkv[:, :C_].astype(ml_dtypes.bfloat16))
    Wv = np.ascontiguousarray(Wkv[:, C_:].astype(ml_dtypes.bfloat16))
    Wq16 = np.ascontiguousarray(Wq.astype(ml_dtypes.bfloat16))
    bq2 = np.ascontiguousarray(bq.reshape(2, 128).T, dtype=np.float32)
    # Wp permuted per otn strip: strip hp rows = [head 2hp (32), bias row,
    # 31 zeros, head 2hp+1 (32), zero, 31 zeros]; the bias row carries
    # bp + bv@Wp (v bias folded through the output projection)
    bv = bkv[C_:]
    bp_eff = bp + bv @ Wp
    wp_perm = np.zeros((4, 128, C_), np.float32)
    for hp in range(4):
        wp_perm[hp, 0:32] = Wp[(2 * hp) * 32:(2 * hp + 1) * 32]
        wp_perm[hp, 64:96] = Wp[(2 * hp + 1) * 32:(2 * hp + 2) * 32]
    wp_perm[0, 32] = bp_eff
    wp_perm = np.ascontiguousarray(wp_perm.astype(ml_dtypes.bfloat16))
    return xT, biasT, emT, mpT, Wq16, Wk, Wv, bq2, wp_perm


def _ensure_ntff_hook():
    import types
    try:
        from antenv import axon_hooks  # noqa: F401
        return
    except ImportError:
        pass
    import antenv
    mod = types.ModuleType("antenv.axon_hooks")
    _h = {"hook": None}
    mod.get_axon_ntff_profile_hook = lambda: _h["hook"]
    mod.set_axon_ntff_profile_hook = lambda hook: _h.__setitem__("hook", hook)
    sys.modules["antenv.axon_hooks"] = mod
    antenv.axon_hooks = mod
    try:
        sys.path.insert(0, "/root/.axon_site/trn_agent_boot")
        import trn_boot
        hook = trn_boot._ntff_profile_via_ctypes("/opt/axon/libaxon_pjrt.so")
        if hook is not None:
            mod.set_axon_ntff_profile_hook(hook)
    except Exception as e:
        print("ntff hook setup failed:", repr(e))


def kernel(x, mask, affine_matrix, Wq, bq, Wkv, bkv, Wp, bp,
           angle_table, H, W, _profile=False):
    if _profile:
        _ensure_ntff_hook()
    x = np.asarray(x, np.float32)
    mask = np.asarray(mask, np.float32)
    affine_matrix = np.asarray(affine_matrix, np.float32)
    Wq = np.asarray(Wq, np.float32); bq = np.asarray(bq, np.float32)
    Wkv = np.asarray(Wkv, np.float32); bkv = np.asarray(bkv, np.float32)
    Wp = np.asarray(Wp, np.float32); bp = np.asarray(bp, np.float32)
    angle_table = np.asarray(angle_table, np.float32)

    (xT, biasT, emT, mpT, Wq16, Wk, Wv, bq2,
     wp_perm) = _host_prep(x, mask, affine_matrix, Wq, bq, Wkv, bkv, Wp, bp,
                           angle_table, H, W)

    if "nc" not in _CACHE:
        _CACHE["nc"] = build_kernel()
    nc = _CACHE["nc"]

    import ml_dtypes
    sel = np.zeros((2, 128), ml_dtypes.bfloat16)
    sel[0, 0:64] = 1.0
    sel[1, 64:128] = 1.0

    in_maps = []
    for m in range(N_CORES):
        bs = [m, m + N_CORES]          # same mask: b % 4 == m % 4
        j = m % NW
        in_maps.append({
            "xT": np.ascontiguousarray(xT[bs]),
            "biasT": np.ascontiguousarray(biasT[bs]),
            "em": emT[j], "mp": mpT[j],
            "wq": Wq16, "wk": Wk, "wv": Wv, "wp": wp_perm,
            "bq": bq2, "sel": sel,
        })

    res = run_bass_kernel_spmd(nc, in_maps, core_ids=list(range(N_CORES)),
                               trace=_profile)
    out = np.empty((B, N, C), np.float32)
    for m in range(N_CORES):
        y = res.results[m]["y"]
        out[m] = y[0]
        out[m + N_CORES] = y[1]
    if _profile:
        return out, res
    return out


if __name__ == "__main__":
    import reference
    inputs = reference.setup_inputs()
    out = kernel(**{k: (np.asarray(v) if hasattr(v, "shape") else v)
                    for k, v in inputs.items()})
    ref = np.asarray(reference.reference(**inputs))
    err = np.abs(out - ref)
    print("max abs err:", err.max(),
          "absmax-rel:", err.max() / np.abs(ref).max())


# revision 9
# speedup vs baseline: 1.1549x; 1.0185x over previous
"""Trainium2 Bass kernel for nn_Attention_10797547782838.

Windowed multi-head attention with per-query angle bias:
  q = (x@Wq+bq) reshaped to heads; k,v = x@Wkv+bkv
  attn = softmax((q*scale)@k^T * anglebias(q) + mask[b%4]); out = attn@v@Wp.

Sharding: batch data-parallel over 8 cores; core m gets batches {m, m+8},
which share the same window mask (b%4 == m%4).

Device algorithm (per batch):
  - S^T layout [keys, queries]; angle bias and scale folded into q on host.
  - k-projection bias is dropped entirely: it contributes a per-query
    constant to the logits, which softmax cancels.  v-projection bias is
    folded into the output-projection bias on host (softmax weights sum
    to 1, so out += bv exactly).
  - exp(S+M) computed two ways to balance engines (chunk-assigned):
      A: ACT exp(S) -> bf16, then multiply by em=exp(M) on DVE or GPSIMD
      B: DVE scalar_tensor_tensor Schraudolph: int16(S*a + M') bitcast bf16
    where M' = rint(a*M + 16256 + c) is an int16 host-precomputed tile, so
    the Schraudolph path gets the mask add for free.  Both A-multiplies and
    the B op work on full [128, 2, 512] chunks with the 512-wide mask
    slice broadcast (stride-0) over the two head halves.
  - PV matmul uses [V_h | 1] 33-wide stationaries: partition 32 of each
    output strip accumulates the softmax denominator.
  - Denominator rows are copied out of PSUM, inverted with
    reciprocal_approx_fast, broadcast across partitions with GPSIMD
    partition_broadcast (no PE involvement), and applied to O^T on DVE
    before the output projection.
  - Output projection uses per-strip Wp with zeroed junk rows so the
    33-partition strips need no compaction.
"""
import math
import sys

import numpy as np

sys.path.insert(0, "/opt/trn_rl_repo")

import concourse.bass as bass  # noqa: E402
import concourse.bacc as bacc  # noqa: E402
import concourse.tile as tile  # noqa: E402
from concourse import mybir  # noqa: E402
from concourse.bass_utils import run_bass_kernel_spmd  # noqa: E402

F32 = mybir.dt.float32
F32R = mybir.dt.float32r
BF16 = mybir.dt.bfloat16
I16 = mybir.dt.int16
AF = mybir.ActivationFunctionType
ALU = mybir.AluOpType

B, N, C = 16, 1024, 256
HEADS, HD = 8, 32
NW = 4
N_CORES = 8
BPC = B // N_CORES  # batches per core

AA = 184.6650390625        # 128 * log2(e)
MAGIC_C = -11.0            # Schraudolph rounding correction

# per-batch chunk counts for exp paths (64 chunks of [128,1024] per batch)
# A: ACT exp + DVE mask-mult; G: ACT exp + GPSIMD mask-mult; B: DVE
# Schraudolph scalar_tensor_tensor (GPSIMD cannot read PSUM)
N_A, N_G, N_B = 19, 24, 21
LAG = 16

_CACHE = {}


def r32(ap):
    return ap.bitcast(F32R)


def bc2(a, n=2):
    # insert a stride-0 broadcast dim after the partition dim: [p, m] -> [p, n, m]
    return bass.AP(a.tensor, a.offset, [a.ap[0], [0, n]] + list(a.ap[1:]))


def make_pattern(na, ng, nb):
    tgt = {"A": na, "G": ng, "B": nb}
    cnt = {"A": 0, "G": 0, "B": 0}
    pat = []
    for _ in range(na + ng + nb):
        best, bscore = None, None
        for p in ("A", "G", "B"):
            if tgt[p] == 0 or cnt[p] >= tgt[p]:
                continue
            score = (cnt[p] + 1.0) / tgt[p]
            if bscore is None or score < bscore:
                best, bscore = p, score
        cnt[best] += 1
        pat.append(best)
    return pat


PATTERN = make_pattern(N_A, N_G, N_B)


def build_kernel():
    from contextlib import ExitStack
    nc = bacc.Bacc("TRN2", target_bir_lowering=False, debug=False,
                   num_devices=N_CORES)

    d_xT = nc.dram_tensor("xT", [BPC, C, N], BF16, kind="ExternalInput").ap()
    d_biasT = nc.dram_tensor("biasT", [BPC, C, N], BF16, kind="ExternalInput").ap()
    d_em = nc.dram_tensor("em", [N, N], BF16, kind="ExternalInput").ap()
    d_mp = nc.dram_tensor("mp", [N, N], I16, kind="ExternalInput").ap()
    d_wq = nc.dram_tensor("wq", [C, C], BF16, kind="ExternalInput").ap()
    d_wk = nc.dram_tensor("wk", [C, C], BF16, kind="ExternalInput").ap()
    d_wv = nc.dram_tensor("wv", [C, C], BF16, kind="ExternalInput").ap()
    d_wp = nc.dram_tensor("wp", [4, 128, C], BF16, kind="ExternalInput").ap()
    d_bq = nc.dram_tensor("bq", [128, 2], F32, kind="ExternalInput").ap()
    d_sel = nc.dram_tensor("sel", [2, 128], BF16, kind="ExternalInput").ap()
    d_y = nc.dram_tensor("y", [BPC, N, C], F32, kind="ExternalOutput").ap()

    with tile.TileContext(nc) as tc:
        with ExitStack() as ctx, nc.allow_low_precision(reason="bf16 matmul inputs; fp32 PSUM accumulation; approx softmax exp within tolerance"):
            kernel_body(ctx, tc, d_xT, d_biasT, d_em, d_mp, d_wq, d_wk, d_wv,
                        d_wp, d_bq, d_sel, d_y)
    nc.compile()
    return nc


def kernel_body(ctx, tc, d_xT, d_biasT, d_em, d_mp, d_wq, d_wk, d_wv, d_wp,
                d_bq, d_sel, d_y):
    nc = tc.nc

    consts = ctx.enter_context(tc.tile_pool(name="consts", bufs=1))
    xpool = ctx.enter_context(tc.tile_pool(name="xpool", bufs=2))
    qkv = ctx.enter_context(tc.tile_pool(name="qkv", bufs=2))
    vpool = ctx.enter_context(tc.tile_pool(name="vpool", bufs=2))
    ppool = ctx.enter_context(tc.tile_pool(name="ppool", bufs=19))
    otpool = ctx.enter_context(tc.tile_pool(name="otpool", bufs=2))
    misc = ctx.enter_context(tc.tile_pool(name="misc", bufs=2))
    ypool = ctx.enter_context(tc.tile_pool(name="ypool", bufs=3))
    psS = ctx.enter_context(tc.tile_pool(name="psS", bufs=2, space="PSUM"))
    psO = ctx.enter_context(tc.tile_pool(name="psO", bufs=1, space="PSUM"))

    # ---- weights + batch-0 inputs first so the first matmuls start早 ----
    w_sb = {}
    w_sb["wq"] = consts.tile([128, 2, C], BF16, tag="w_wq", name="w_wq")
    nc.sync.dma_start(out=w_sb["wq"][:],
                      in_=d_wq.rearrange("(c p) n -> p c n", p=128))
    bq_sb = consts.tile([128, 2], F32, tag="bq")
    nc.sync.dma_start(out=bq_sb[:], in_=d_bq[:])

    xT_sb, biasT_sb, qT_sb, kT_sb, v_sb = {}, {}, {}, {}, {}
    for b in range(BPC):
        xT_sb[b] = xpool.tile([128, 2, N], BF16, tag="xT", name=f"xT{b}")
        nc.sync.dma_start(out=xT_sb[b][:],
                          in_=d_xT[b].rearrange("(c p) n -> p c n", p=128))
        biasT_sb[b] = xpool.tile([128, 2, N], BF16, tag="biasT", name=f"bT{b}")
        nc.sync.dma_start(out=biasT_sb[b][:],
                          in_=d_biasT[b].rearrange("(c p) n -> p c n", p=128))
        if b == 0:
            w_sb["wk"] = consts.tile([128, 2, C], BF16, tag="w_wk", name="w_wk")
            nc.sync.dma_start(out=w_sb["wk"][:],
                              in_=d_wk.rearrange("(c p) n -> p c n", p=128))
            w_sb["wv"] = consts.tile([128, 2, C], BF16, tag="w_wv", name="w_wv")
            nc.sync.dma_start(out=w_sb["wv"][:],
                              in_=d_wv.rearrange("(c p) n -> p c n", p=128))

    # masks staged once per core (both batches share b%4)
    em_sb = [consts.tile([128, N], BF16, tag=f"em{kc}", name=f"em{kc}")
             for kc in range(8)]
    mp_sb = [consts.tile([128, N], I16, tag=f"mp{kc}", name=f"mp{kc}")
             for kc in range(8)]
    for kc in range(8):
        nc.sync.dma_start(out=em_sb[kc][:], in_=d_em[kc * 128:(kc + 1) * 128, :])
        nc.sync.dma_start(out=mp_sb[kc][:], in_=d_mp[kc * 128:(kc + 1) * 128, :])

    # wp comes pre-permuted per otn strip: [4 strips, 128 rows, C]
    # (strip 0 row 32 carries bp + bv@Wp so no separate bias add is needed)
    wp_sb = consts.tile([128, 4, C], BF16, tag="w_wp")
    nc.sync.dma_start(out=wp_sb[:], in_=d_wp.rearrange("s p n -> p s n"))
    # selector rows: selA -> partitions 0:64, selB -> partitions 64:128
    selA = consts.tile([1, 128], BF16, tag="selA")
    nc.sync.dma_start(out=selA[:], in_=d_sel[0:1, :])
    selB = consts.tile([1, 128], BF16, tag="selB")
    nc.sync.dma_start(out=selB[:], in_=d_sel[1:2, :])

    # ---- projections for both batches ----
    for b in range(BPC):
        qT_sb[b] = qkv.tile([128, 2, N], BF16, tag="qT", name=f"qT{b}")
        kT_sb[b] = qkv.tile([128, 2, N], BF16, tag="kT", name=f"kT{b}")
        for m in range(2):
            for qc in range(2):
                ps_q = psS.tile([128, 512], F32, tag="s")
                for ci in range(2):
                    nc.tensor.matmul(
                        ps_q[:],
                        w_sb["wq"][:, ci, m * 128:(m + 1) * 128],
                        xT_sb[b][:, ci, qc * 512:(qc + 1) * 512],
                        start=(ci == 0), stop=(ci == 1))
                nc.vector.scalar_tensor_tensor(
                    out=qT_sb[b][:, m, qc * 512:(qc + 1) * 512], in0=ps_q[:],
                    scalar=bq_sb[:, m:m + 1],
                    in1=biasT_sb[b][:, m, qc * 512:(qc + 1) * 512],
                    op0=ALU.add, op1=ALU.mult)
                ps_k = psS.tile([128, 512], F32, tag="s")
                for ci in range(2):
                    nc.tensor.matmul(
                        ps_k[:],
                        w_sb["wk"][:, ci, m * 128:(m + 1) * 128],
                        xT_sb[b][:, ci, qc * 512:(qc + 1) * 512],
                        start=(ci == 0), stop=(ci == 1))
                # k bias dropped: softmax cancels per-query logit offsets
                keng = nc.scalar if (m + qc) % 2 == 0 else nc.vector
                if keng is nc.scalar:
                    keng.copy(
                        out=kT_sb[b][:, m, qc * 512:(qc + 1) * 512], in_=ps_k[:])
                else:
                    keng.tensor_scalar_add(
                        out=kT_sb[b][:, m, qc * 512:(qc + 1) * 512],
                        in0=ps_k[:], scalar1=0.0)

        v_sb[b] = []
        for t8 in range(8):
            vt = vpool.tile([128, 8, 33], BF16, tag=f"v{t8}", name=f"v{b}_{t8}")
            ps_v = psS.tile([128, C], F32, tag="s")
            for ci in range(2):
                nc.tensor.matmul(
                    ps_v[:],
                    xT_sb[b][:, ci, t8 * 128:(t8 + 1) * 128],
                    w_sb["wv"][:, ci, :],
                    start=(ci == 0), stop=(ci == 1))
            # bv folded into wp bias row on host
            if t8 % 2 == 0:
                nc.scalar.copy(out=vt[:, :, 0:32], in_=ps_v[:])
            else:
                nc.vector.tensor_scalar_add(out=vt[:, :, 0:32], in0=ps_v[:],
                                            scalar1=0.0)
            nc.gpsimd.memset(vt[:, :, 32:33], 1.0)
            v_sb[b].append(vt)

    # ---- attention: batches interleaved; PV matmuls and tail pieces are
    # ---- emitted with a lag so the in-order PE queue never head-of-line
    # ---- blocks on the exp/mask path of the current chunk
    from collections import deque
    pending = deque()
    tail_state = {}
    tail_todo = []

    def step(fn):
        pending.append(fn)
        while len(pending) > LAG:
            pending.popleft()()

    chunk_i = {0: 0, 1: 0}
    for qc in range(2):
        for b in range(BPC):
            pot = psO.tile([128, 4, 512], F32, tag="po")
            for kc in range(8):
                for hp in range(4):  # head pair (2hp, 2hp+1)
                    # spread the previous superstep's tail pieces between
                    # chunks (through the deque, so they stay ordered after
                    # that superstep's deferred PV matmuls) so the in-order
                    # PE queue never head-of-line blocks on the denominator
                    # chain
                    ci_step = kc * 4 + hp
                    if ci_step in (0, 3, 6, 9) and tail_todo:
                        pending.append(tail_todo.pop(0))
                    path = PATTERN[chunk_i[b]]
                    chunk_i[b] += 1
                    ps_s = psS.tile([128, N], F32, tag="s")
                    for hh in range(2):
                        h = 2 * hp + hh
                        j = h % 4
                        nc.tensor.matmul(
                            ps_s[:, hh * 512:(hh + 1) * 512],
                            kT_sb[b][32 * j:32 * (j + 1), h // 4,
                                     kc * 128:(kc + 1) * 128],
                            qT_sb[b][32 * j:32 * (j + 1), h // 4,
                                     qc * 512:(qc + 1) * 512],
                            start=True, stop=True,
                            tile_position=(32 * j, 0))
                    emsl = em_sb[kc][:, qc * 512:(qc + 1) * 512]
                    mpsl = mp_sb[kc][:, qc * 512:(qc + 1) * 512]
                    if path in ("A", "G"):
                        p0 = ppool.tile([128, N], BF16, tag="p0")
                        pt = ppool.tile([128, N], BF16, tag="pt")
                        nc.scalar.activation(out=p0[:], in_=ps_s[:], func=AF.Exp)
                        eng = nc.vector if path == "A" else nc.gpsimd
                        eng.tensor_tensor(
                            out=pt[:].rearrange("p (h q) -> p h q", h=2),
                            in0=p0[:].rearrange("p (h q) -> p h q", h=2),
                            in1=bc2(emsl), op=ALU.mult)
                        pt_bf = pt
                    else:
                        pti = ppool.tile([128, N], I16, tag="pt")
                        nc.vector.scalar_tensor_tensor(
                            out=pti[:].rearrange("p (h q) -> p h q", h=2),
                            in0=ps_s[:].rearrange("p (h q) -> p h q", h=2),
                            scalar=AA, in1=bc2(mpsl),
                            op0=ALU.mult, op1=ALU.add)
                        pt_bf = pti.bitcast(BF16)

                    def po_mms(pot=pot, b=b, kc=kc, hp=hp, pt_bf=pt_bf):
                        for hh in range(2):
                            h = 2 * hp + hh
                            nc.tensor.matmul(
                                pot[64 * hh:64 * hh + 33, hp, :],
                                v_sb[b][kc][:, h, :],
                                pt_bf[:, hh * 512:(hh + 1) * 512],
                                start=(kc == 0), stop=(kc == 7),
                                tile_position=(0, 64 * hh))
                    step(po_mms)

            otn = [None] * 4

            def tail_a(pot=pot, otn=otn):
                # gather the denominator rows (partitions 32, 96) as bf16,
                # broadcast across partitions via 1-row selector matmuls
                # (rows 0-63 head 2hp, rows 64-127 head 2hp+1), invert
                # straight out of PSUM, apply to the O^T strips (hp 0,1)
                rs0 = misc.tile([1, 4, 512], BF16, tag="rs0")
                nc.scalar.copy(out=rs0[:], in_=pot[32:33, :, :])
                rs1 = misc.tile([1, 4, 512], BF16, tag="rs1")
                nc.vector.tensor_scalar_add(out=rs1[:],
                                            in0=pot[96:97, :, :], scalar1=0.0)
                ps_rb = psS.tile([128, 2, 512], F32, tag="s")
                for j in range(2):
                    nc.tensor.matmul(ps_rb[:, j, :], selA[:], rs0[:, j, :],
                                     start=True, stop=False)
                    nc.tensor.matmul(ps_rb[:, j, :], selB[:], rs1[:, j, :],
                                     start=False, stop=True)
                rbs = misc.tile([128, 2, 512], F32, tag="rbs")
                nc.vector.reciprocal_approx_fast(out=rbs[:], in_=ps_rb[:])
                tail_state[(id(pot), 'a')] = (rs0, rs1, rbs)
                for hp in range(2):
                    ot = otpool.tile([128, 512], BF16, tag=f"otn{hp}",
                                     name=f"ot{hp}")
                    nc.vector.tensor_tensor(
                        out=ot[:], in0=pot[:, hp, :], in1=rbs[:, hp, :],
                        op=ALU.mult)
                    otn[hp] = ot

            def tail_b(pot=pot, otn=otn):
                rs0, rs1, _ = tail_state[(id(pot), 'a')]
                ps_rb = psS.tile([128, 2, 512], F32, tag="s")
                for j in range(2):
                    nc.tensor.matmul(ps_rb[:, j, :], selA[:], rs0[:, 2 + j, :],
                                     start=True, stop=False)
                    nc.tensor.matmul(ps_rb[:, j, :], selB[:], rs1[:, 2 + j, :],
                                     start=False, stop=True)
                rbs = misc.tile([128, 2, 512], F32, tag="rbs")
                nc.vector.reciprocal_approx_fast(out=rbs[:], in_=ps_rb[:])
                for hp in range(2, 4):
                    ot = otpool.tile([128, 512], BF16, tag=f"otn{hp}",
                                     name=f"ot{hp}")
                    nc.vector.tensor_tensor(
                        out=ot[:], in0=pot[:, hp, :], in1=rbs[:, hp - 2, :],
                        op=ALU.mult)
                    otn[hp] = ot

            def tail_y(qp, b=b, qc=qc, otn=otn):
                # two q-tiles of the output projection per call
                ps_y = psS.tile([128, 2, C], F32, tag="s")
                for j in range(2):
                    qt = 2 * qp + j
                    for hp in range(4):
                        nc.tensor.matmul(
                            ps_y[:, j, :],
                            otn[hp][:, qt * 128:(qt + 1) * 128],
                            wp_sb[:, hp, :],
                            start=(hp == 0), stop=(hp == 3))
                y_sb = ypool.tile([128, 2, C], F32, tag="y")
                nc.scalar.copy(out=y_sb[:], in_=ps_y[:])
                qoff = qc * 512 + qp * 256
                nc.sync.dma_start(
                    out=d_y[b, qoff:qoff + 256, :].rearrange(
                        "(a p) n -> p a n", p=128),
                    in_=y_sb[:])

            # flush any tail pieces still owed, then shrink the deque so
            # this superstep's deferred PVs finish popping early in the
            # next superstep, leaving room to spread the new tail pieces
            while tail_todo:
                pending.append(tail_todo.pop(0))
            while len(pending) > 8:
                pending.popleft()()
            tail_todo = [tail_a, tail_b,
                         lambda f=tail_y: f(0), lambda f=tail_y: f(1)]

    while tail_todo:
        pending.append(tail_todo.pop(0))
    while pending:
        pending.popleft()()


def _host_prep(x, mask, affine_matrix, Wq, bq, Wkv, bkv, Wp, bp,
               angle_table, H, W):
    B_, N_, C_ = x.shape
    heads = angle_table.shape[1]
    hd = C_ // heads
    scale = np.float32(hd ** -0.5)
    H = int(H); W = int(W)

    gy, gx = np.meshgrid(np.arange(H, dtype=np.float32),
                         np.arange(W, dtype=np.float32), indexing="ij")
    coords = np.stack([gx.reshape(-1), gy.reshape(-1)], -1).astype(np.float32)
    center = np.array([W / 2.0, H / 2.0], np.float32)
    ego = np.einsum("bij,j->bi", affine_matrix[:, :2, :2], center) \
        + affine_matrix[:, :2, 2]
    rel = coords[None, :, :] - ego[:, None, :]
    ang = np.arctan2(rel[..., 1], rel[..., 0]).astype(np.float32)
    bins = (((ang + np.float32(math.pi)) / np.float32(2.0 * math.pi))
            * (angle_table.shape[0] - 1)).astype(np.int32)
    sig = (1.0 / (1.0 + np.exp(-angle_table[bins]))).astype(np.float32)
    bias = (1.0 + sig).astype(np.float32)                      # (B,N,h)

    import ml_dtypes
    biasT = np.repeat(bias.transpose(0, 2, 1) * scale, hd, axis=1)  # (B,C,N)
    biasT = np.ascontiguousarray(biasT).astype(ml_dtypes.bfloat16)
    xT = np.ascontiguousarray(
        x.transpose(0, 2, 1).astype(ml_dtypes.bfloat16))
    maskT = mask.transpose(0, 2, 1)                                # [nW,k,q]
    emT = np.ascontiguousarray(np.exp(maskT).astype(ml_dtypes.bfloat16))
    mpT = np.ascontiguousarray(
        np.rint(np.float64(AA) * maskT + (127 * 128 + MAGIC_C))
        .astype(np.int16))

    Wk = np.ascontiguousarray(Wkv[:, :C_].astype(ml_dtypes.bfloat16))
    Wv = np.ascontiguousarray(Wkv[:, C_:].astype(ml_dtypes.bfloat16))
    Wq16 = np.ascontiguousarray(Wq.astype(ml_dtypes.bfloat16))
    bq2 = np.ascontiguousarray(bq.reshape(2, 128).T, dtype=np.float32)
    # Wp permuted per otn strip: strip hp rows = [head 2hp (32), bias row,
    # 31 zeros, head 2hp+1 (32), zero, 31 zeros]; the bias row carries
    # bp + bv@Wp (v bias folded through the output projection)
    bv = bkv[C_:]
    bp_eff = bp + bv @ Wp
    wp_perm = np.zeros((4, 128, C_), np.float32)
    for hp in range(4):
        wp_perm[hp, 0:32] = Wp[(2 * hp) * 32:(2 * hp + 1) * 32]
        wp_perm[hp, 64:96] = Wp[(2 * hp + 1) * 32:(2 * hp + 2) * 32]
    wp_perm[0, 32] = bp_eff
    wp_perm = np.ascontiguousarray(wp_perm.astype(ml_dtypes.bfloat16))
    return xT, biasT, emT, mpT, Wq16, Wk, Wv, bq2, wp_perm


def _ensure_ntff_hook():
    import types
    try:
        from antenv import axon_hooks  # noqa: F401
        return
    except ImportError:
        pass
    import antenv
    mod = types.ModuleType("antenv.axon_hooks")
    _h = {"hook": None}
    mod.get_axon_ntff_profile_hook = lambda: _h["hook"]
    mod.set_axon_ntff_profile_hook = lambda hook: _h.__setitem__("hook", hook)
    sys.modules["antenv.axon_hooks"] = mod
    antenv.axon_hooks = mod
    try:
        sys.path.insert(0, "/root/.axon_site/trn_agent_boot")
        import trn_boot
        hook = trn_boot._ntff_profile_via_ctypes("/opt/axon/libaxon_pjrt.so")
        if hook is not None:
            mod.set_axon_ntff_profile_hook(hook)
    except Exception as e:
        print("ntff hook setup failed:", repr(e))


def kernel(x, mask, affine_matrix, Wq, bq, Wkv, bkv, Wp, bp,
           angle_table, H, W, _profile=False):
    if _profile:
        _ensure_ntff_hook()
    x = np.asarray(x, np.float32)
    mask = np.asarray(mask, np.float32)
    affine_matrix = np.asarray(affine_matrix, np.float32)
    Wq = np.asarray(Wq, np.float32); bq = np.asarray(bq, np.float32)
    Wkv = np.asarray(Wkv, np.float32); bkv = np.asarray(bkv, np.float32)
    Wp = np.asarray(Wp, np.float32); bp = np.asarray(bp, np.float32)
    angle_table = np.asarray(angle_table, np.float32)

    (xT, biasT, emT, mpT, Wq16, Wk, Wv, bq2,
     wp_perm) = _host_prep(x, mask, affine_matrix, Wq, bq, Wkv, bkv, Wp, bp,
                           angle_table, H, W)

    if "nc" not in _CACHE:
        _CACHE["nc"] = build_kernel()
    nc = _CACHE["nc"]

    import ml_dtypes
    sel = np.zeros((2, 128), ml_dtypes.bfloat16)
    sel[0, 0:64] = 1.0
    sel[1, 64:128] = 1.0

    in_maps = []
    for m in range(N_CORES):
        bs = [m, m + N_CORES]          # same mask: b % 4 == m % 4
        j = m % NW
        in_maps.append({
            "xT": np.ascontiguousarray(xT[bs]),
            "biasT": np.ascontiguousarray(biasT[bs]),
            "em": emT[j], "mp": mpT[j],
            "wq": Wq16, "wk": Wk, "wv": Wv, "wp": wp_perm,
            "bq": bq2, "sel": sel,
        })

    res = run_bass_kernel_spmd(nc, in_maps, core_ids=list(range(N_CORES)),
                               trace=_profile)
    out = np.empty((B, N, C), np.float32)
    for m in range(N_CORES):
        y = res.results[m]["y"]
        out[m] = y[0]
        out[m + N_CORES] = y[1]
    if _profile:
        return out, res
    return out


if __name__ == "__main__":
    import reference
    inputs = reference.setup_inputs()
    out = kernel(**{k: (np.asarray(v) if hasattr(v, "shape") else v)
                    for k, v in inputs.items()})
    ref = np.asarray(reference.reference(**inputs))
    err = np.abs(out - ref)
    print("max abs err:", err.max(),
          "absmax-rel:", err.max() / np.abs(ref).max())
